# revision 17
# baseline (speedup 1.0000x reference)
"""Trainium2 Bass kernel for ChemicalSpecialist2D (3-layer gated GNN).

Strategy (8 NeuronCores, SPMD):
  - Nodes are range-sharded: core k owns rows [2500k, 2500k+2500) and all
    edges whose destination (row) falls in that range.
  - The edge MLP is algebraically factored through node-level matmuls:
        pre_e = A[row_e] + B[col_e] + Cp[type_e],   A = h @ W1a, B = h @ W1b,
        Cp[t] = bond_table[t] @ W1c + b1
        u_e  = relu(pre_e) * gate_e          (gate > 0 so relu(x)*g == relu(x*g))
        S    = segment_sum(u, row);  h += S @ W2 + gate_sums (x) b2
    This turns the per-edge [E,576]@[576,256] matmul into two node-level
    [N,256]@[256,256] matmuls plus per-edge gathers/adds.
  - Each core computes A/B for its node slice from an SBUF-resident h^T;
    B slices are AllGathered into a shared table each layer; per-edge rows are
    fetched with indirect DMA (B bypass, A and Cp with accumulate), gated+relu'd
    on the scalar engine, and scatter-added into 128-node windows with one-hot
    matmuls on the tensor engine (PSUM accumulation).
  - The final property MLP runs node-parallel on-device; the tiny functional-
    group readout, embeddings, and output assembly are host-side.
"""

import numpy as np

N = 20000
E_TOT = 320000
H = 256
BD = 64
N_CORES = 8
NLOC = N // N_CORES          # 2500
WIN = 128                    # nodes per scatter window
NWIN = (NLOC + WIN - 1) // WIN   # 20
NPAD = NWIN * WIN            # 2560 padded nodes per core
P = 128

_CACHE = {}


def _build_host_data(inputs):
    x = np.asarray(inputs["x"], np.float32)
    edge_index = np.asarray(inputs["edge_index"]).astype(np.int64)
    edge_attr = np.asarray(inputs["edge_attr"], np.float32)
    atom_table = np.asarray(inputs["atom_table"], np.float32)
    charge_table = np.asarray(inputs["charge_table"], np.float32)
    hybrid_table = np.asarray(inputs["hybrid_table"], np.float32)
    bond_table = np.asarray(inputs["bond_table"], np.float32)
    gnn_w1 = np.asarray(inputs["gnn_w1"], np.float32)
    gnn_b1 = np.asarray(inputs["gnn_b1"], np.float32)
    gnn_w2 = np.asarray(inputs["gnn_w2"], np.float32)
    gnn_b2 = np.asarray(inputs["gnn_b2"], np.float32)
    att_w = np.asarray(inputs["att_w"], np.float32)
    att_b = np.asarray(inputs["att_b"], np.float32)

    atom_types = np.clip(x[:, 0].astype(np.int32), 0, 10)
    formal_charges = np.clip(x[:, 2].astype(np.int32) + 3, 0, 6)
    hybridization = np.clip(x[:, 1].astype(np.int32), 0, 7)
    bond_types = np.clip(edge_attr[:, 0].astype(np.int32), 0, 4)

    h0 = np.zeros((N, H), np.float32)
    h0[:, 0:64] = atom_table[atom_types]
    h0[:, 64:96] = charge_table[formal_charges]
    h0[:, 96:128] = hybrid_table[hybridization]

    # per-layer weight repack
    w1a = np.ascontiguousarray(gnn_w1[:, 0:256, :])        # [3,256,256]
    w1b = np.ascontiguousarray(gnn_w1[:, 256:512, :])      # [3,256,256]
    w1c = gnn_w1[:, 512:576, :]                            # [3,64,256]
    cp = np.einsum("tb,lbo->lto", bond_table, w1c) + gnn_b1[:, None, :]  # [3,5,256]
    w2 = np.ascontiguousarray(gnn_w2)                      # [3,256,256]
    b2 = np.ascontiguousarray(gnn_b2)                      # [3,256]
    gate_vals = 1.0 / (1.0 + np.exp(-(np.einsum("tb,lbo->lto", bond_table, att_w)
                                      + att_b[:, None, :])))  # [3,5,1]
    gate_vals = gate_vals[:, :, 0]                         # [3,5]

    row = edge_index[0]
    col = edge_index[1]
    core_of = row // NLOC

    # global max tiles per (core, window)
    t_w = 0
    per_core = []
    for k in range(N_CORES):
        sel = np.nonzero(core_of == k)[0]
        rloc = (row[sel] - k * NLOC).astype(np.int64)
        wid = rloc // WIN
        order = np.argsort(wid, kind="stable")
        sel, rloc, wid = sel[order], rloc[order], wid[order]
        counts = np.bincount(wid, minlength=NWIN)
        t_w = max(t_w, int(np.ceil(counts.max() / P)))
        per_core.append((sel, rloc, wid, counts))

    data = []
    for k in range(N_CORES):
        sel, rloc, wid, counts = per_core[k]
        bt_k = bond_types[sel]
        col_k = col[sel].astype(np.int64)

        boffs = np.zeros((NWIN, P, t_w), np.int32)
        aoffs = np.zeros((NWIN, P, t_w), np.int32)
        roww = np.full((NWIN, P, t_w), -1.0, np.float32)
        gate = np.zeros((3, NWIN, P, t_w), np.float32)
        gsum = np.zeros((3, NPAD), np.float32)

        start = 0
        for w in range(NWIN):
            c = int(counts[w])
            e = slice(start, start + c)
            idx = np.arange(c)
            pp, tt = idx % P, idx // P
            boffs[w, pp, tt] = ((col_k[e] // NLOC) * NPAD + col_k[e] % NLOC)
            aoffs[w, pp, tt] = rloc[e] * 5 + bt_k[e]
            roww[w, pp, tt] = (rloc[e] - w * WIN).astype(np.float32)
            for l in range(3):
                gate[l, w, pp, tt] = gate_vals[l][bt_k[e]]
            start += c

        for l in range(3):
            gsum[l, :NLOC] = np.bincount(rloc, weights=gate_vals[l][bt_k],
                                         minlength=NLOC)

        h0T = np.zeros((2, P, NPAD), np.float32)
        hs = h0[k * NLOC:(k + 1) * NLOC]           # [2500, 256]
        h0T[0, :, :NLOC] = hs[:, 0:128].T
        h0T[1, :, :NLOC] = hs[:, 128:256].T

        data.append(dict(
            hT_in=h0T,
            boffs=boffs.transpose(1, 0, 2).reshape(P, NWIN * t_w),
            aoffs=aoffs.transpose(1, 0, 2).reshape(P, NWIN * t_w),
            roww=roww.transpose(1, 0, 2).reshape(P, NWIN * t_w),
            gate=gate.transpose(0, 2, 1, 3).reshape(3, P, NWIN * t_w),
            gsum=gsum.reshape(3, 1, NPAD),
        ))

    shared = dict(
        w1a=w1a, w1b=w1b, w2=w2,
        b2=b2.reshape(3, 1, 256), cp=cp,
        prop_w1=np.asarray(inputs["prop_w1"], np.float32),
        prop_b1=np.asarray(inputs["prop_b1"], np.float32).reshape(128, 1),
        prop_w2=np.asarray(inputs["prop_w2"], np.float32),
        prop_b2=np.asarray(inputs["prop_b2"], np.float32).reshape(32, 1),
    )
    aux = dict(atom_types=atom_types,
               fg_w=np.asarray(inputs["fg_w"], np.float32),
               fg_b=np.asarray(inputs["fg_b"], np.float32))
    return t_w, data, shared, aux


def _build_program(t_w):
    import concourse.bacc as bacc
    import concourse.mybir as mybir
    import concourse.tile as tile
    from concourse.bass import IndirectOffsetOnAxis
    from concourse.masks import make_identity
    from concourse._compat import get_trn_type

    f32 = mybir.dt.float32
    i32 = mybir.dt.int32

    nc = bacc.Bacc(get_trn_type() or "TRN2", target_bir_lowering=False,
                   debug=False, num_devices=N_CORES)

    # ---- I/O tensors -----------------------------------------------------
    hT_in = nc.dram_tensor("hT_in", [2, P, NPAD], f32, kind="ExternalInput")
    w1a_t = nc.dram_tensor("w1a", [3, 256, 256], f32, kind="ExternalInput")
    w1b_t = nc.dram_tensor("w1b", [3, 256, 256], f32, kind="ExternalInput")
    w2_t = nc.dram_tensor("w2", [3, 256, 256], f32, kind="ExternalInput")
    b2_t = nc.dram_tensor("b2", [3, 1, 256], f32, kind="ExternalInput")
    cp_t = nc.dram_tensor("cp", [3, 5, 256], f32, kind="ExternalInput")
    boffs_t = nc.dram_tensor("boffs", [P, NWIN * t_w], i32, kind="ExternalInput")
    aoffs_t = nc.dram_tensor("aoffs", [P, NWIN * t_w], i32, kind="ExternalInput")
    roww_t = nc.dram_tensor("roww", [P, NWIN * t_w], f32, kind="ExternalInput")
    gate_t = nc.dram_tensor("gate", [3, P, NWIN * t_w], f32, kind="ExternalInput")
    gsum_t = nc.dram_tensor("gsum", [3, 1, NPAD], f32, kind="ExternalInput")
    pw1_t = nc.dram_tensor("prop_w1", [256, 128], f32, kind="ExternalInput")
    pb1_t = nc.dram_tensor("prop_b1", [128, 1], f32, kind="ExternalInput")
    pw2_t = nc.dram_tensor("prop_w2", [128, 32], f32, kind="ExternalInput")
    pb2_t = nc.dram_tensor("prop_b2", [32, 1], f32, kind="ExternalInput")

    out_hT = nc.dram_tensor("out_hT", [2, P, NPAD], f32, kind="ExternalOutput")
    out_pT = nc.dram_tensor("out_pT", [32, NPAD], f32, kind="ExternalOutput")

    # internal DRAM
    TA = nc.dram_tensor("TA", [NPAD * 5, 256], f32)         # (A_local[i] + Cp[t]) at row i*5+t
    Bloc = nc.dram_tensor("Bloc", [NPAD, 256], f32)          # AllGather input
    TB = nc.dram_tensor("TB", [N_CORES * NPAD, 256], f32, addr_space="Shared")

    with tile.TileContext(nc) as tc:
        with (
            tc.tile_pool(name="const", bufs=1) as cpool,
            tc.tile_pool(name="hst", bufs=1) as hpool,
            tc.tile_pool(name="wts", bufs=2) as wpool,
            tc.tile_pool(name="ab", bufs=4) as abpool,
            tc.tile_pool(name="offs", bufs=3) as opool,
            tc.tile_pool(name="edge", bufs=6) as epool,
            tc.tile_pool(name="small", bufs=6) as spool,
            tc.tile_pool(name="psA", bufs=1, space="PSUM") as psA,
            tc.tile_pool(name="psS", bufs=2, space="PSUM") as psS,
            tc.tile_pool(name="psT", bufs=2, space="PSUM") as psT,
            tc.tile_pool(name="psU", bufs=1, space="PSUM") as psU,
        ):
            # constants
            iota_i = cpool.tile([P, P], i32)
            nc.gpsimd.iota(iota_i[:], pattern=[[1, P]], base=0, channel_multiplier=0)
            iota_f = cpool.tile([P, P], f32)
            nc.vector.tensor_copy(iota_f[:], iota_i[:])
            ident = cpool.tile([P, P], f32)
            make_identity(nc, ident)
            ones_sb = cpool.tile([1, P], f32)
            nc.vector.memset(ones_sb[:], 1.0)

            # resident h^T and S^T
            hT = [hpool.tile([P, NPAD], f32, tag=f"hT{h}", name=f"hT{h}") for h in range(2)]
            ST = [hpool.tile([P, NPAD], f32, tag=f"ST{h}", name=f"ST{h}") for h in range(2)]
            for h in range(2):
                nc.sync.dma_start(hT[h][:], hT_in.ap()[h])

            for l in range(3):
                # ---- phase A: A_local / B_local, AllGather ----------------
                w1a_sb = [wpool.tile([P, 256], f32, tag=f"w1a{h}", name=f"w1a{h}") for h in range(2)]
                w1b_sb = [wpool.tile([P, 256], f32, tag=f"w1b{h}", name=f"w1b{h}") for h in range(2)]
                for h in range(2):
                    nc.sync.dma_start(w1a_sb[h][:], w1a_t.ap()[l, 128 * h:128 * (h + 1), :])
                    nc.sync.dma_start(w1b_sb[h][:], w1b_t.ap()[l, 128 * h:128 * (h + 1), :])
                cp_sb = spool.tile([1, 5 * 256], f32, tag="cp_sb", bufs=2)
                nc.sync.dma_start(cp_sb[:], cp_t.ap().rearrange("l t d -> l (t d)")[l:l + 1, :])
                cpb = wpool.tile([P, 5 * 256], f32, tag="cpb")
                for t in range(5):
                    pc = psT.tile([P, 256], f32, tag="pT", name="pc")
                    nc.tensor.matmul(pc[:], lhsT=ones_sb[0:1, :],
                                     rhs=cp_sb[0:1, t * 256:(t + 1) * 256],
                                     start=True, stop=True)
                    nc.vector.tensor_copy(cpb[:, t * 256:(t + 1) * 256], pc[:])

                for it in range(NWIN):
                    sl = slice(it * P, (it + 1) * P)
                    pb = psA.tile([P, 256], f32, tag="pb")
                    for h in range(2):
                        nc.tensor.matmul(pb[:], lhsT=hT[h][:, sl], rhs=w1b_sb[h][:],
                                         start=(h == 0), stop=(h == 1))
                    sb = abpool.tile([P, 256], f32, tag="sb")
                    nc.vector.tensor_copy(sb[:], pb[:])
                    nc.sync.dma_start(Bloc.ap()[sl, :], sb[:])

                nc.gpsimd.collective_compute(
                    "AllGather", mybir.AluOpType.bypass,
                    replica_groups=[list(range(N_CORES))],
                    ins=[Bloc.ap()], outs=[TB.ap()],
                )

                for it in range(NWIN):
                    sl = slice(it * P, (it + 1) * P)
                    pa = psA.tile([P, 256], f32, tag="pa")
                    for h in range(2):
                        nc.tensor.matmul(pa[:], lhsT=hT[h][:, sl], rhs=w1a_sb[h][:],
                                         start=(h == 0), stop=(h == 1))
                    for t in range(5):
                        sa = abpool.tile([P, 256], f32, tag="sa", name=f"sa{t}")
                        nc.vector.tensor_tensor(out=sa[:], in0=pa[:],
                                                in1=cpb[:, t * 256:(t + 1) * 256],
                                                op=mybir.AluOpType.add)
                        dst = TA.ap().rearrange("(i five) d -> five i d", five=5)
                        nc.sync.dma_start(dst[t, it * P:(it + 1) * P, :], sa[:])

                # ---- phase B: edge loop -----------------------------------
                for w in range(NWIN):
                    bo = opool.tile([P, t_w], i32, tag="bo")
                    ao = opool.tile([P, t_w], i32, tag="ao")
                    nc.sync.dma_start(bo[:], boffs_t.ap()[:, w * t_w:(w + 1) * t_w])
                    nc.sync.dma_start(ao[:], aoffs_t.ap()[:, w * t_w:(w + 1) * t_w])
                    rw = spool.tile([P, t_w], f32, tag="rw")
                    gt = spool.tile([P, t_w], f32, tag="gt")
                    nc.sync.dma_start(rw[:], roww_t.ap()[:, w * t_w:(w + 1) * t_w])
                    nc.sync.dma_start(gt[:], gate_t.ap()[l, :, w * t_w:(w + 1) * t_w])

                    pS = psS.tile([P, 256], f32, tag="pS")
                    for t in range(t_w):
                        preB = epool.tile([P, 256], f32, tag="preB")
                        preA = epool.tile([P, 256], f32, tag="preA")
                        nc.gpsimd.indirect_dma_start(
                            out=preB[:], out_offset=None, in_=TB.ap(),
                            in_offset=IndirectOffsetOnAxis(ap=bo[:, t:t + 1], axis=0))
                        nc.gpsimd.indirect_dma_start(
                            out=preA[:], out_offset=None, in_=TA.ap(),
                            in_offset=IndirectOffsetOnAxis(ap=ao[:, t:t + 1], axis=0))
                        pre = epool.tile([P, 256], f32, tag="pre")
                        nc.vector.tensor_tensor(out=pre[:], in0=preB[:], in1=preA[:],
                                                op=mybir.AluOpType.add)
                        oh = spool.tile([P, P], f32, tag="oh")
                        nc.vector.tensor_scalar(oh[:], iota_f[:], rw[:, t:t + 1],
                                                None, mybir.AluOpType.is_equal)
                        u = spool.tile([P, 256], f32, tag="u")
                        nc.scalar.activation(u[:], pre[:],
                                             mybir.ActivationFunctionType.Relu,
                                             scale=gt[:, t:t + 1])
                        nc.tensor.matmul(pS[:], lhsT=oh[:], rhs=u[:],
                                         start=(t == 0), stop=(t == t_w - 1))
                    s_sb = spool.tile([P, 256], f32, tag="s_sb")
                    nc.vector.tensor_copy(s_sb[:], pS[:])
                    for h in range(2):
                        pT = psT.tile([P, P], f32, tag="pT")
                        nc.tensor.transpose(pT[:], s_sb[:, 128 * h:128 * (h + 1)], ident[:])
                        nc.vector.tensor_copy(ST[h][:, w * P:(w + 1) * P], pT[:])

                # ---- phase C: h^T += W2^T @ S^T + b2 (x) gsum -------------
                w2_sb = [wpool.tile([P, 256], f32, tag=f"w2{h}", name=f"w2{h}") for h in range(2)]
                for h in range(2):
                    nc.sync.dma_start(w2_sb[h][:], w2_t.ap()[l, 128 * h:128 * (h + 1), :])
                b2_sb = spool.tile([1, 256], f32, tag="b2", bufs=2)
                nc.sync.dma_start(b2_sb[:], b2_t.ap()[l])
                g_sb = spool.tile([1, NPAD], f32, tag="g", bufs=2)
                nc.sync.dma_start(g_sb[:], gsum_t.ap()[l])

                for ho in range(2):
                    osl = slice(128 * ho, 128 * (ho + 1))
                    for c0 in range(0, NPAD, 512):
                        csz = min(512, NPAD - c0)
                        csl = slice(c0, c0 + csz)
                        pU = psU.tile([P, 512], f32, tag="big")
                        for hi in range(2):
                            nc.tensor.matmul(pU[:, :csz], lhsT=w2_sb[hi][:, osl],
                                             rhs=ST[hi][:, csl],
                                             start=(hi == 0), stop=False)
                        nc.tensor.matmul(pU[:, :csz], lhsT=b2_sb[0:1, osl],
                                         rhs=g_sb[0:1, csl], start=False, stop=True)
                        nc.vector.tensor_tensor(out=hT[ho][:, csl], in0=hT[ho][:, csl],
                                                in1=pU[:, :csz], op=mybir.AluOpType.add)

            # ---- epilogue: property MLP + outputs -------------------------
            pw1_sb = [wpool.tile([P, 128], f32, tag=f"pw1{h}", name=f"pw1{h}") for h in range(2)]
            for h in range(2):
                nc.sync.dma_start(pw1_sb[h][:], pw1_t.ap()[128 * h:128 * (h + 1), :])
            pw2_sb = wpool.tile([P, 32], f32, tag="pw2")
            nc.sync.dma_start(pw2_sb[:], pw2_t.ap())
            pb1_sb = spool.tile([P, 1], f32, tag="pb1", bufs=1)
            nc.sync.dma_start(pb1_sb[:], pb1_t.ap())
            pb2_sb = spool.tile([32, 1], f32, tag="pb2", bufs=1)
            nc.sync.dma_start(pb2_sb[:], pb2_t.ap())
            propsT = hpool.tile([32, NPAD], f32, tag="propsT")

            for c0 in range(0, NPAD, 512):
                csz = min(512, NPAD - c0)
                csl = slice(c0, c0 + csz)
                p1 = psU.tile([P, 512], f32, tag="big", name="p1")
                for h in range(2):
                    nc.tensor.matmul(p1[:, :csz], lhsT=pw1_sb[h][:], rhs=hT[h][:, csl],
                                     start=(h == 0), stop=(h == 1))
                u1 = spool.tile([P, 512], f32, tag="u1", bufs=2)
                nc.scalar.activation(u1[:, :csz], p1[:, :csz],
                                     mybir.ActivationFunctionType.Relu,
                                     bias=pb1_sb[:, 0:1])
                p2 = psU.tile([32, 512], f32, tag="p2")
                nc.tensor.matmul(p2[:, :csz], lhsT=pw2_sb[:], rhs=u1[:, :csz],
                                 start=True, stop=True)
                nc.scalar.activation(propsT[:, csl], p2[:, :csz],
                                     mybir.ActivationFunctionType.Identity,
                                     bias=pb2_sb[:, 0:1])

            for h in range(2):
                nc.sync.dma_start(out_hT.ap()[h], hT[h][:])
            nc.sync.dma_start(out_pT.ap(), propsT[:])

    nc.compile()
    return nc


def _get_runner(t_w):
    """Build the Bass program once and wrap it in a reusable jitted callable
    (mirrors bass2jax.run_bass_via_pjrt's multi-core branch, but keeps the
    jitted function so repeat calls skip tracing/compile)."""
    key = t_w
    if key in _CACHE:
        return _CACHE[key]
    nc = _build_program(t_w)

    import jax
    import numpy as _np
    from jax.sharding import Mesh, PartitionSpec
    from jax.experimental.shard_map import shard_map
    import concourse.mybir as mybir
    from concourse import bass2jax
    from concourse.bass2jax import _bass_exec_p, partition_id_tensor

    bass2jax.install_neuronx_cc_hook()

    in_names, out_names, out_avals, zero_shapes = [], [], [], []
    partition_name = nc.partition_id_tensor.name if nc.partition_id_tensor else None
    for alloc in nc.m.functions[0].allocations:
        if not isinstance(alloc, mybir.MemoryLocationSet):
            continue
        name = alloc.memorylocations[0].name
        if alloc.kind == "ExternalInput":
            if name != partition_name:
                in_names.append(name)
        elif alloc.kind == "ExternalOutput":
            shape = tuple(alloc.tensor_shape)
            dtype = mybir.dt.np(alloc.dtype)
            out_names.append(name)
            out_avals.append(jax.core.ShapedArray(shape, dtype))
            zero_shapes.append((shape, dtype))
    n_params = len(in_names)
    all_names = list(in_names) + list(out_names)
    if partition_name is not None:
        all_names.append(partition_name)

    def _body(*args):
        operands = list(args)
        if partition_name is not None:
            operands.append(partition_id_tensor())
        outs = _bass_exec_p.bind(
            *operands,
            out_avals=tuple(out_avals),
            in_names=tuple(all_names),
            out_names=tuple(out_names),
            lowering_input_output_aliases=(),
            sim_require_finite=True,
            sim_require_nnan=True,
            nc=nc,
        )
        return tuple(outs)

    devices = jax.devices()[:N_CORES]
    mesh = Mesh(_np.asarray(devices), ("core",))
    in_specs = (PartitionSpec("core"),) * (n_params + len(out_names))
    out_specs = (PartitionSpec("core"),) * len(out_names)
    sharded = jax.jit(
        shard_map(_body, mesh=mesh, in_specs=in_specs, out_specs=out_specs,
                  check_rep=False),
        keep_unused=True,
    )

    def run(in_maps, timing=None):
        concat_in = [
            np.concatenate([np.asarray(in_maps[c][nm]) for c in range(N_CORES)], axis=0)
            for nm in in_names
        ]
        concat_zeros = [np.zeros((N_CORES * s[0], *s[1:]), d) for s, d in zero_shapes]
        args = [jax.device_put(a) for a in concat_in + concat_zeros]
        for a in args:
            a.block_until_ready()
        import time as _time
        t0 = _time.perf_counter()
        out_arrs = sharded(*args)
        jax.block_until_ready(out_arrs)
        t1 = _time.perf_counter()
        if timing is not None:
            timing.append(t1 - t0)
        return [
            {nm: np.asarray(out_arrs[i]).reshape(N_CORES, *out_avals[i].shape)[c]
             for i, nm in enumerate(out_names)}
            for c in range(N_CORES)
        ]

    _CACHE[key] = run
    return run


def kernel(**inputs):
    t_w, data, shared, aux = _build_host_data(inputs)
    run = _get_runner(t_w)

    in_maps = []
    for k in range(N_CORES):
        d = data[k]
        m = dict(
            hT_in=d["hT_in"],
            boffs=d["boffs"], aoffs=d["aoffs"],
            roww=d["roww"], gate=d["gate"], gsum=d["gsum"],
            w1a=shared["w1a"], w1b=shared["w1b"], w2=shared["w2"],
            b2=shared["b2"], cp=shared["cp"],
            prop_w1=shared["prop_w1"], prop_b1=shared["prop_b1"],
            prop_w2=shared["prop_w2"], prop_b2=shared["prop_b2"],
        )
        in_maps.append(m)

    timing = []
    results = run(in_maps, timing=timing)
    kernel.last_exec_s = timing[0] if timing else None

    h_full = np.zeros((N, H), np.float32)
    props = np.zeros((N, 32), np.float32)
    for k in range(N_CORES):
        hT = results[k]["out_hT"]          # [2,128,NPAD]
        pT = results[k]["out_pT"]          # [32,NPAD]
        sl = slice(k * NLOC, (k + 1) * NLOC)
        h_full[sl, 0:128] = hT[0][:, :NLOC].T
        h_full[sl, 128:256] = hT[1][:, :NLOC].T
        props[sl] = pT[:, :NLOC].T

    pooled = h_full.mean(axis=0)
    fg = (np.einsum("d,kdo->ko", pooled, aux["fg_w"]) + aux["fg_b"]).reshape(-1)
    fg_features = np.broadcast_to(fg.astype(np.float32), (N, 64)).copy()
    return h_full, props, fg_features, aux["atom_types"]


# revision 19
# speedup vs baseline: 1.3315x; 1.3315x over previous
"""Trainium2 Bass kernel for ChemicalSpecialist2D (3-layer gated GNN).

Strategy (8 NeuronCores, SPMD):
  - Nodes are range-sharded: core k owns rows [2500k, 2500k+2500) and all
    edges whose destination (row) falls in that range.
  - The edge MLP is algebraically factored through node-level matmuls:
        pre_e = A[row_e] + B[col_e] + Cp[type_e],   A = h @ W1a, B = h @ W1b,
        Cp[t] = bond_table[t] @ W1c + b1
        u_e  = relu(pre_e) * gate_e          (gate > 0 so relu(x)*g == relu(x*g))
        S    = segment_sum(u, row);  h += S @ W2 + gate_sums (x) b2
    This turns the per-edge [E,576]@[576,256] matmul into two node-level
    [N,256]@[256,256] matmuls plus per-edge gathers/adds.
  - Each core computes A/B for its node slice from an SBUF-resident h^T;
    B slices are AllGathered into a shared table each layer; per-edge rows are
    fetched with indirect DMA (B bypass, A and Cp with accumulate), gated+relu'd
    on the scalar engine, and scatter-added into 128-node windows with one-hot
    matmuls on the tensor engine (PSUM accumulation).
  - The final property MLP runs node-parallel on-device; the tiny functional-
    group readout, embeddings, and output assembly are host-side.
"""

import numpy as np

N = 20000
E_TOT = 320000
H = 256
BD = 64
N_CORES = 8
NLOC = N // N_CORES          # 2500
WIN = 128                    # nodes per scatter window
NWIN = (NLOC + WIN - 1) // WIN   # 20
NPAD = NWIN * WIN            # 2560 padded nodes per core
P = 128

_CACHE = {}


def _build_host_data(inputs):
    x = np.asarray(inputs["x"], np.float32)
    edge_index = np.asarray(inputs["edge_index"]).astype(np.int64)
    edge_attr = np.asarray(inputs["edge_attr"], np.float32)
    atom_table = np.asarray(inputs["atom_table"], np.float32)
    charge_table = np.asarray(inputs["charge_table"], np.float32)
    hybrid_table = np.asarray(inputs["hybrid_table"], np.float32)
    bond_table = np.asarray(inputs["bond_table"], np.float32)
    gnn_w1 = np.asarray(inputs["gnn_w1"], np.float32)
    gnn_b1 = np.asarray(inputs["gnn_b1"], np.float32)
    gnn_w2 = np.asarray(inputs["gnn_w2"], np.float32)
    gnn_b2 = np.asarray(inputs["gnn_b2"], np.float32)
    att_w = np.asarray(inputs["att_w"], np.float32)
    att_b = np.asarray(inputs["att_b"], np.float32)

    atom_types = np.clip(x[:, 0].astype(np.int32), 0, 10)
    formal_charges = np.clip(x[:, 2].astype(np.int32) + 3, 0, 6)
    hybridization = np.clip(x[:, 1].astype(np.int32), 0, 7)
    bond_types = np.clip(edge_attr[:, 0].astype(np.int32), 0, 4)

    h0 = np.zeros((N, H), np.float32)
    h0[:, 0:64] = atom_table[atom_types]
    h0[:, 64:96] = charge_table[formal_charges]
    h0[:, 96:128] = hybrid_table[hybridization]

    # per-layer weight repack
    w1a = np.ascontiguousarray(gnn_w1[:, 0:256, :])        # [3,256,256]
    w1b = np.ascontiguousarray(gnn_w1[:, 256:512, :])      # [3,256,256]
    w1c = gnn_w1[:, 512:576, :]                            # [3,64,256]
    cp = np.einsum("tb,lbo->lto", bond_table, w1c) + gnn_b1[:, None, :]  # [3,5,256]
    w2 = np.ascontiguousarray(gnn_w2)                      # [3,256,256]
    b2 = np.ascontiguousarray(gnn_b2)                      # [3,256]
    gate_vals = 1.0 / (1.0 + np.exp(-(np.einsum("tb,lbo->lto", bond_table, att_w)
                                      + att_b[:, None, :])))  # [3,5,1]
    gate_vals = gate_vals[:, :, 0]                         # [3,5]

    row = edge_index[0]
    col = edge_index[1]
    core_of = row // NLOC

    # global max tiles per (core, window)
    t_w = 0
    per_core = []
    for k in range(N_CORES):
        sel = np.nonzero(core_of == k)[0]
        rloc = (row[sel] - k * NLOC).astype(np.int64)
        wid = rloc // WIN
        order = np.argsort(wid, kind="stable")
        sel, rloc, wid = sel[order], rloc[order], wid[order]
        counts = np.bincount(wid, minlength=NWIN)
        t_w = max(t_w, int(np.ceil(counts.max() / P)))
        per_core.append((sel, rloc, wid, counts))

    data = []
    for k in range(N_CORES):
        sel, rloc, wid, counts = per_core[k]
        bt_k = bond_types[sel]
        col_k = col[sel].astype(np.int64)

        boffs = np.zeros((NWIN, P, t_w), np.int32)
        aoffs = np.zeros((NWIN, P, t_w), np.int32)
        roww = np.full((NWIN, P, t_w), -1.0, np.float32)
        gate = np.zeros((3, NWIN, P, t_w), np.float32)
        gsum = np.zeros((3, NPAD), np.float32)

        start = 0
        for w in range(NWIN):
            c = int(counts[w])
            e = slice(start, start + c)
            idx = np.arange(c)
            pp, tt = idx % P, idx // P
            boffs[w, pp, tt] = ((col_k[e] // NLOC) * NPAD + col_k[e] % NLOC)
            aoffs[w, pp, tt] = rloc[e] * 5 + bt_k[e]
            roww[w, pp, tt] = (rloc[e] - w * WIN).astype(np.float32)
            for l in range(3):
                gate[l, w, pp, tt] = gate_vals[l][bt_k[e]]
            start += c

        for l in range(3):
            gsum[l, :NLOC] = np.bincount(rloc, weights=gate_vals[l][bt_k],
                                         minlength=NLOC)

        h0T = np.zeros((2, P, NPAD), np.float32)
        hs = h0[k * NLOC:(k + 1) * NLOC]           # [2500, 256]
        h0T[0, :, :NLOC] = hs[:, 0:128].T
        h0T[1, :, :NLOC] = hs[:, 128:256].T

        data.append(dict(
            hT_in=h0T,
            boffs=boffs.transpose(1, 0, 2).reshape(P, NWIN * t_w),
            aoffs=aoffs.transpose(1, 0, 2).reshape(P, NWIN * t_w),
            roww=roww.transpose(1, 0, 2).reshape(P, NWIN * t_w),
            gate=gate.transpose(0, 2, 1, 3).reshape(3, P, NWIN * t_w),
            gsum=gsum.reshape(3, 1, NPAD),
        ))

    shared = dict(
        w1a=w1a, w1b=w1b, w2=w2,
        b2=b2.reshape(3, 1, 256), cp=cp,
        prop_w1=np.asarray(inputs["prop_w1"], np.float32),
        prop_b1=np.asarray(inputs["prop_b1"], np.float32).reshape(128, 1),
        prop_w2=np.asarray(inputs["prop_w2"], np.float32),
        prop_b2=np.asarray(inputs["prop_b2"], np.float32).reshape(32, 1),
    )
    aux = dict(atom_types=atom_types,
               fg_w=np.asarray(inputs["fg_w"], np.float32),
               fg_b=np.asarray(inputs["fg_b"], np.float32))
    return t_w, data, shared, aux


def _build_program(t_w):
    import concourse.bacc as bacc
    import concourse.mybir as mybir
    import concourse.tile as tile
    from concourse.bass import IndirectOffsetOnAxis
    from concourse.masks import make_identity
    from concourse._compat import get_trn_type

    f32 = mybir.dt.float32
    i32 = mybir.dt.int32

    nc = bacc.Bacc(get_trn_type() or "TRN2", target_bir_lowering=False,
                   debug=False, num_devices=N_CORES)

    # ---- I/O tensors -----------------------------------------------------
    hT_in = nc.dram_tensor("hT_in", [2, P, NPAD], f32, kind="ExternalInput")
    w1a_t = nc.dram_tensor("w1a", [3, 256, 256], f32, kind="ExternalInput")
    w1b_t = nc.dram_tensor("w1b", [3, 256, 256], f32, kind="ExternalInput")
    w2_t = nc.dram_tensor("w2", [3, 256, 256], f32, kind="ExternalInput")
    b2_t = nc.dram_tensor("b2", [3, 1, 256], f32, kind="ExternalInput")
    cp_t = nc.dram_tensor("cp", [3, 5, 256], f32, kind="ExternalInput")
    boffs_t = nc.dram_tensor("boffs", [P, NWIN * t_w], i32, kind="ExternalInput")
    aoffs_t = nc.dram_tensor("aoffs", [P, NWIN * t_w], i32, kind="ExternalInput")
    roww_t = nc.dram_tensor("roww", [P, NWIN * t_w], f32, kind="ExternalInput")
    gate_t = nc.dram_tensor("gate", [3, P, NWIN * t_w], f32, kind="ExternalInput")
    gsum_t = nc.dram_tensor("gsum", [3, 1, NPAD], f32, kind="ExternalInput")
    pw1_t = nc.dram_tensor("prop_w1", [256, 128], f32, kind="ExternalInput")
    pb1_t = nc.dram_tensor("prop_b1", [128, 1], f32, kind="ExternalInput")
    pw2_t = nc.dram_tensor("prop_w2", [128, 32], f32, kind="ExternalInput")
    pb2_t = nc.dram_tensor("prop_b2", [32, 1], f32, kind="ExternalInput")

    out_hT = nc.dram_tensor("out_hT", [2, P, NPAD], f32, kind="ExternalOutput")
    out_pT = nc.dram_tensor("out_pT", [32, NPAD], f32, kind="ExternalOutput")

    # internal DRAM
    TA = nc.dram_tensor("TA", [NPAD * 5, 256], f32)         # (A_local[i] + Cp[t]) at row i*5+t
    Bloc = nc.dram_tensor("Bloc", [NPAD, 256], f32)          # AllGather input
    TB = nc.dram_tensor("TB", [N_CORES * NPAD, 256], f32, addr_space="Shared")

    with tile.TileContext(nc) as tc:
        with (
            tc.tile_pool(name="const", bufs=1) as cpool,
            tc.tile_pool(name="hst", bufs=1) as hpool,
            tc.tile_pool(name="wts", bufs=2) as wpool,
            tc.tile_pool(name="ab", bufs=4) as abpool,
            tc.tile_pool(name="offs", bufs=6) as opool,
            tc.tile_pool(name="edge", bufs=10) as epool,
            tc.tile_pool(name="small", bufs=6) as spool,
            tc.tile_pool(name="psA", bufs=1, space="PSUM") as psA,
            tc.tile_pool(name="psS", bufs=2, space="PSUM") as psS,
            tc.tile_pool(name="psT", bufs=2, space="PSUM") as psT,
            tc.tile_pool(name="psU", bufs=1, space="PSUM") as psU,
        ):
            # constants
            iota_i = cpool.tile([P, P], i32)
            nc.gpsimd.iota(iota_i[:], pattern=[[1, P]], base=0, channel_multiplier=0)
            iota_f = cpool.tile([P, P], f32)
            nc.vector.tensor_copy(iota_f[:], iota_i[:])
            ident = cpool.tile([P, P], f32)
            make_identity(nc, ident)
            ones_sb = cpool.tile([1, P], f32)
            nc.vector.memset(ones_sb[:], 1.0)

            # resident h^T and S^T
            hT = [hpool.tile([P, NPAD], f32, tag=f"hT{h}", name=f"hT{h}") for h in range(2)]
            ST = [hpool.tile([P, NPAD], f32, tag=f"ST{h}", name=f"ST{h}") for h in range(2)]
            for h in range(2):
                nc.sync.dma_start(hT[h][:], hT_in.ap()[h])

            for l in range(3):
                # ---- phase A: A_local / B_local, AllGather ----------------
                w1a_sb = [wpool.tile([P, 256], f32, tag=f"w1a{h}", name=f"w1a{h}") for h in range(2)]
                w1b_sb = [wpool.tile([P, 256], f32, tag=f"w1b{h}", name=f"w1b{h}") for h in range(2)]
                for h in range(2):
                    nc.sync.dma_start(w1a_sb[h][:], w1a_t.ap()[l, 128 * h:128 * (h + 1), :])
                    nc.sync.dma_start(w1b_sb[h][:], w1b_t.ap()[l, 128 * h:128 * (h + 1), :])
                cp_sb = spool.tile([1, 5 * 256], f32, tag="cp_sb", bufs=2)
                nc.sync.dma_start(cp_sb[:], cp_t.ap().rearrange("l t d -> l (t d)")[l:l + 1, :])
                cpb = wpool.tile([P, 5 * 256], f32, tag="cpb")
                for t in range(5):
                    pc = psT.tile([P, 256], f32, tag="pT", name="pc")
                    nc.tensor.matmul(pc[:], lhsT=ones_sb[0:1, :],
                                     rhs=cp_sb[0:1, t * 256:(t + 1) * 256],
                                     start=True, stop=True)
                    nc.vector.tensor_copy(cpb[:, t * 256:(t + 1) * 256], pc[:])

                for it in range(NWIN):
                    sl = slice(it * P, (it + 1) * P)
                    pb = psA.tile([P, 256], f32, tag="pb")
                    for h in range(2):
                        nc.tensor.matmul(pb[:], lhsT=hT[h][:, sl], rhs=w1b_sb[h][:],
                                         start=(h == 0), stop=(h == 1))
                    sb = abpool.tile([P, 256], f32, tag="sb")
                    nc.vector.tensor_copy(sb[:], pb[:])
                    nc.sync.dma_start(Bloc.ap()[sl, :], sb[:])

                nc.gpsimd.collective_compute(
                    "AllGather", mybir.AluOpType.bypass,
                    replica_groups=[list(range(N_CORES))],
                    ins=[Bloc.ap()], outs=[TB.ap()],
                )

                for it in range(NWIN):
                    sl = slice(it * P, (it + 1) * P)
                    pa = psA.tile([P, 256], f32, tag="pa")
                    for h in range(2):
                        nc.tensor.matmul(pa[:], lhsT=hT[h][:, sl], rhs=w1a_sb[h][:],
                                         start=(h == 0), stop=(h == 1))
                    for t in range(5):
                        sa = abpool.tile([P, 256], f32, tag="sa", name=f"sa{t}")
                        nc.vector.tensor_tensor(out=sa[:], in0=pa[:],
                                                in1=cpb[:, t * 256:(t + 1) * 256],
                                                op=mybir.AluOpType.add)
                        dst = TA.ap().rearrange("(i five) d -> five i d", five=5)
                        nc.sync.dma_start(dst[t, it * P:(it + 1) * P, :], sa[:])

                # ---- phase B: edge loop -----------------------------------
                for w in range(NWIN):
                    bo = opool.tile([P, t_w], i32, tag="bo")
                    ao = opool.tile([P, t_w], i32, tag="ao")
                    nc.sync.dma_start(bo[:], boffs_t.ap()[:, w * t_w:(w + 1) * t_w])
                    nc.sync.dma_start(ao[:], aoffs_t.ap()[:, w * t_w:(w + 1) * t_w])
                    rw = spool.tile([P, t_w], f32, tag="rw")
                    gt = spool.tile([P, t_w], f32, tag="gt")
                    nc.sync.dma_start(rw[:], roww_t.ap()[:, w * t_w:(w + 1) * t_w])
                    nc.sync.dma_start(gt[:], gate_t.ap()[l, :, w * t_w:(w + 1) * t_w])

                    pS = psS.tile([P, 256], f32, tag="pS")
                    for t in range(t_w):
                        pre = epool.tile([P, 256], f32, tag="pre")
                        # A first: local table, independent of the AllGather, so
                        # these gathers overlap the collective; B then adds in.
                        nc.gpsimd.indirect_dma_start(
                            out=pre[:], out_offset=None, in_=TA.ap(),
                            in_offset=IndirectOffsetOnAxis(ap=ao[:, t:t + 1], axis=0))
                        nc.gpsimd.indirect_dma_start(
                            out=pre[:], out_offset=None, in_=TB.ap(),
                            in_offset=IndirectOffsetOnAxis(ap=bo[:, t:t + 1], axis=0),
                            compute_op=mybir.AluOpType.add)
                        oh = spool.tile([P, P], f32, tag="oh")
                        nc.vector.tensor_scalar(oh[:], iota_f[:], rw[:, t:t + 1],
                                                None, mybir.AluOpType.is_equal)
                        u = spool.tile([P, 256], f32, tag="u")
                        nc.scalar.activation(u[:], pre[:],
                                             mybir.ActivationFunctionType.Relu,
                                             scale=gt[:, t:t + 1])
                        nc.tensor.matmul(pS[:], lhsT=oh[:], rhs=u[:],
                                         start=(t == 0), stop=(t == t_w - 1))
                    s_sb = spool.tile([P, 256], f32, tag="s_sb")
                    nc.vector.tensor_copy(s_sb[:], pS[:])
                    for h in range(2):
                        pT = psT.tile([P, P], f32, tag="pT")
                        nc.tensor.transpose(pT[:], s_sb[:, 128 * h:128 * (h + 1)], ident[:])
                        nc.vector.tensor_copy(ST[h][:, w * P:(w + 1) * P], pT[:])

                # ---- phase C: h^T += W2^T @ S^T + b2 (x) gsum -------------
                w2_sb = [wpool.tile([P, 256], f32, tag=f"w2{h}", name=f"w2{h}") for h in range(2)]
                for h in range(2):
                    nc.sync.dma_start(w2_sb[h][:], w2_t.ap()[l, 128 * h:128 * (h + 1), :])
                b2_sb = spool.tile([1, 256], f32, tag="b2", bufs=2)
                nc.sync.dma_start(b2_sb[:], b2_t.ap()[l])
                g_sb = spool.tile([1, NPAD], f32, tag="g", bufs=2)
                nc.sync.dma_start(g_sb[:], gsum_t.ap()[l])

                for ho in range(2):
                    osl = slice(128 * ho, 128 * (ho + 1))
                    for c0 in range(0, NPAD, 512):
                        csz = min(512, NPAD - c0)
                        csl = slice(c0, c0 + csz)
                        pU = psU.tile([P, 512], f32, tag="big")
                        for hi in range(2):
                            nc.tensor.matmul(pU[:, :csz], lhsT=w2_sb[hi][:, osl],
                                             rhs=ST[hi][:, csl],
                                             start=(hi == 0), stop=False)
                        nc.tensor.matmul(pU[:, :csz], lhsT=b2_sb[0:1, osl],
                                         rhs=g_sb[0:1, csl], start=False, stop=True)
                        nc.vector.tensor_tensor(out=hT[ho][:, csl], in0=hT[ho][:, csl],
                                                in1=pU[:, :csz], op=mybir.AluOpType.add)

            # ---- epilogue: property MLP + outputs -------------------------
            pw1_sb = [wpool.tile([P, 128], f32, tag=f"pw1{h}", name=f"pw1{h}") for h in range(2)]
            for h in range(2):
                nc.sync.dma_start(pw1_sb[h][:], pw1_t.ap()[128 * h:128 * (h + 1), :])
            pw2_sb = wpool.tile([P, 32], f32, tag="pw2")
            nc.sync.dma_start(pw2_sb[:], pw2_t.ap())
            pb1_sb = spool.tile([P, 1], f32, tag="pb1", bufs=1)
            nc.sync.dma_start(pb1_sb[:], pb1_t.ap())
            pb2_sb = spool.tile([32, 1], f32, tag="pb2", bufs=1)
            nc.sync.dma_start(pb2_sb[:], pb2_t.ap())
            propsT = hpool.tile([32, NPAD], f32, tag="propsT")

            for c0 in range(0, NPAD, 512):
                csz = min(512, NPAD - c0)
                csl = slice(c0, c0 + csz)
                p1 = psU.tile([P, 512], f32, tag="big", name="p1")
                for h in range(2):
                    nc.tensor.matmul(p1[:, :csz], lhsT=pw1_sb[h][:], rhs=hT[h][:, csl],
                                     start=(h == 0), stop=(h == 1))
                u1 = spool.tile([P, 512], f32, tag="u1", bufs=2)
                nc.scalar.activation(u1[:, :csz], p1[:, :csz],
                                     mybir.ActivationFunctionType.Relu,
                                     bias=pb1_sb[:, 0:1])
                p2 = psU.tile([32, 512], f32, tag="p2")
                nc.tensor.matmul(p2[:, :csz], lhsT=pw2_sb[:], rhs=u1[:, :csz],
                                 start=True, stop=True)
                nc.scalar.activation(propsT[:, csl], p2[:, :csz],
                                     mybir.ActivationFunctionType.Identity,
                                     bias=pb2_sb[:, 0:1])

            for h in range(2):
                nc.sync.dma_start(out_hT.ap()[h], hT[h][:])
            nc.sync.dma_start(out_pT.ap(), propsT[:])

    nc.compile()
    return nc


def _get_runner(t_w):
    """Build the Bass program once and wrap it in a reusable jitted callable
    (mirrors bass2jax.run_bass_via_pjrt's multi-core branch, but keeps the
    jitted function so repeat calls skip tracing/compile)."""
    key = t_w
    if key in _CACHE:
        return _CACHE[key]
    nc = _build_program(t_w)

    import jax
    import numpy as _np
    from jax.sharding import Mesh, PartitionSpec
    from jax.experimental.shard_map import shard_map
    import concourse.mybir as mybir
    from concourse import bass2jax
    from concourse.bass2jax import _bass_exec_p, partition_id_tensor

    bass2jax.install_neuronx_cc_hook()

    in_names, out_names, out_avals, zero_shapes = [], [], [], []
    partition_name = nc.partition_id_tensor.name if nc.partition_id_tensor else None
    for alloc in nc.m.functions[0].allocations:
        if not isinstance(alloc, mybir.MemoryLocationSet):
            continue
        name = alloc.memorylocations[0].name
        if alloc.kind == "ExternalInput":
            if name != partition_name:
                in_names.append(name)
        elif alloc.kind == "ExternalOutput":
            shape = tuple(alloc.tensor_shape)
            dtype = mybir.dt.np(alloc.dtype)
            out_names.append(name)
            out_avals.append(jax.core.ShapedArray(shape, dtype))
            zero_shapes.append((shape, dtype))
    n_params = len(in_names)
    all_names = list(in_names) + list(out_names)
    if partition_name is not None:
        all_names.append(partition_name)

    def _body(*args):
        operands = list(args)
        if partition_name is not None:
            operands.append(partition_id_tensor())
        outs = _bass_exec_p.bind(
            *operands,
            out_avals=tuple(out_avals),
            in_names=tuple(all_names),
            out_names=tuple(out_names),
            lowering_input_output_aliases=(),
            sim_require_finite=True,
            sim_require_nnan=True,
            nc=nc,
        )
        return tuple(outs)

    devices = jax.devices()[:N_CORES]
    mesh = Mesh(_np.asarray(devices), ("core",))
    in_specs = (PartitionSpec("core"),) * (n_params + len(out_names))
    out_specs = (PartitionSpec("core"),) * len(out_names)
    sharded = jax.jit(
        shard_map(_body, mesh=mesh, in_specs=in_specs, out_specs=out_specs,
                  check_rep=False),
        keep_unused=True,
    )

    def run(in_maps, timing=None):
        concat_in = [
            np.concatenate([np.asarray(in_maps[c][nm]) for c in range(N_CORES)], axis=0)
            for nm in in_names
        ]
        concat_zeros = [np.zeros((N_CORES * s[0], *s[1:]), d) for s, d in zero_shapes]
        args = [jax.device_put(a) for a in concat_in + concat_zeros]
        for a in args:
            a.block_until_ready()
        import time as _time
        t0 = _time.perf_counter()
        out_arrs = sharded(*args)
        jax.block_until_ready(out_arrs)
        t1 = _time.perf_counter()
        if timing is not None:
            timing.append(t1 - t0)
        return [
            {nm: np.asarray(out_arrs[i]).reshape(N_CORES, *out_avals[i].shape)[c]
             for i, nm in enumerate(out_names)}
            for c in range(N_CORES)
        ]

    _CACHE[key] = run
    return run


def kernel(**inputs):
    t_w, data, shared, aux = _build_host_data(inputs)
    run = _get_runner(t_w)

    in_maps = []
    for k in range(N_CORES):
        d = data[k]
        m = dict(
            hT_in=d["hT_in"],
            boffs=d["boffs"], aoffs=d["aoffs"],
            roww=d["roww"], gate=d["gate"], gsum=d["gsum"],
            w1a=shared["w1a"], w1b=shared["w1b"], w2=shared["w2"],
            b2=shared["b2"], cp=shared["cp"],
            prop_w1=shared["prop_w1"], prop_b1=shared["prop_b1"],
            prop_w2=shared["prop_w2"], prop_b2=shared["prop_b2"],
        )
        in_maps.append(m)

    timing = []
    results = run(in_maps, timing=timing)
    kernel.last_exec_s = timing[0] if timing else None

    h_full = np.zeros((N, H), np.float32)
    props = np.zeros((N, 32), np.float32)
    for k in range(N_CORES):
        hT = results[k]["out_hT"]          # [2,128,NPAD]
        pT = results[k]["out_pT"]          # [32,NPAD]
        sl = slice(k * NLOC, (k + 1) * NLOC)
        h_full[sl, 0:128] = hT[0][:, :NLOC].T
        h_full[sl, 128:256] = hT[1][:, :NLOC].T
        props[sl] = pT[:, :NLOC].T

    pooled = h_full.mean(axis=0)
    fg = (np.einsum("d,kdo->ko", pooled, aux["fg_w"]) + aux["fg_b"]).reshape(-1)
    fg_features = np.broadcast_to(fg.astype(np.float32), (N, 64)).copy()
    return h_full, props, fg_features, aux["atom_types"]
